# revision 1
# baseline (speedup 1.0000x reference)
"""Multi-head causal attention (B=4, T=2048, C=1024, H=16, HD=64) on 8 TRN2 NeuronCores.

Sharding: core c handles batch b = c//2 and heads hg*8..hg*8+8 where hg = c%2
(data parallel on B, tensor parallel on heads). Each core computes
qkv projection for its head group, causal attention for its 8 heads, and a
partial output projection over its 512 local channels. Host sums the two
partial projections per batch and adds the bias.

Per-core device layouts (all chosen so no on-chip transposes of x/W are needed):
  xT    [C=1024, T=2048] bf16   (x[b].T, host-transposed)
  wqkT  [C=1024, 1024]   bf16   (rows: Q of 8 heads then K of 8 heads, transposed;
                                 Q part pre-scaled by HD^-0.5)
  wvT   [C=1024, 512]    bf16
  wpT   [512, 1024]      bf16   (W_proj columns for local channels, transposed)
Stages:
  1. qkT[o, t] = W_sel @ x.T  (o: 512 Q + 512 K, head-pair h'=2p at partitions
     0-63 / 64-127 of chunk p)  and V[t, d] natural.
  2. Per head pair, per 512-row i-super: S[i, j] = Q K^T via row-tiled (K=64)
     matmuls, causal mask add on diagonal 128-blocks, exp on ACT with row-sum
     accumulation, 1/s row scale on DVE, P -> P.T via DMA transpose (bf16),
     then O.T[d, i] accumulated over j-tiles with col-tiled (M=64) matmuls.
  3. y[t, :] = O @ WpT (bf16) accumulated in fp32 PSUM.
"""

import numpy as np
import ml_dtypes

B, T, C = 4, 2048, 1024
H = 16
HD = 64
NCORES = 8
P = 128

_CACHE = {}


def _build_program():
    import concourse.bass as bass
    import concourse.mybir as mybir
    import concourse.tile as tile
    from concourse import bacc
    from concourse.masks import make_causal_mask
    from contextlib import ExitStack

    DT_BF = mybir.dt.bfloat16
    DT_F32 = mybir.dt.float32
    Exp = mybir.ActivationFunctionType.Exp
    X = mybir.AxisListType.X

    nc = bacc.Bacc("TRN2", target_bir_lowering=False, num_devices=NCORES)
    xT = nc.dram_tensor("xT", [C, T], DT_BF, kind="ExternalInput")
    wqkT = nc.dram_tensor("wqkT", [C, 1024], DT_BF, kind="ExternalInput")
    wvT = nc.dram_tensor("wvT", [C, 512], DT_BF, kind="ExternalInput")
    wpT = nc.dram_tensor("wpT", [512, 1024], DT_BF, kind="ExternalInput")
    y = nc.dram_tensor("y", [T, C], DT_F32, kind="ExternalOutput")

    NT = T // P            # 16 t-tiles
    NSUP = 4               # i-supers of 512
    NPAIR = 4              # head pairs per core

    with tile.TileContext(nc) as tc, ExitStack() as ctx:
        pers = ctx.enter_context(tc.tile_pool(name="pers", bufs=1))
        workp = ctx.enter_context(tc.tile_pool(name="workp", bufs=6))
        # xt halves and P.T buffers share 4 rotating 16KB/partition slots
        pt4 = ctx.enter_context(tc.tile_pool(name="pt4", bufs=4))
        worky = ctx.enter_context(tc.tile_pool(name="worky", bufs=2))
        small = ctx.enter_context(tc.tile_pool(name="small", bufs=12))
        sps = ctx.enter_context(tc.tile_pool(name="sps", bufs=2, space="PSUM"))
        av_ps = ctx.enter_context(tc.tile_pool(name="av", bufs=1, space="PSUM"))
        acc_ps = ctx.enter_context(tc.tile_pool(name="acc", bufs=3, space="PSUM"))

        # ---- loads ----
        xt0 = pt4.tile([P, 4, T], DT_BF, tag="pt4")
        xt1 = pt4.tile([P, 4, T], DT_BF, tag="pt4")
        xtre = xT.rearrange("(o p) t -> p o t", p=P)

        def xt(cc):
            return (xt0 if cc < 4 else xt1)[:, cc % 4, :]
        wqk = pers.tile([P, 8, 1024], DT_BF, tag="wqk")
        wv = pers.tile([P, 8, 512], DT_BF, tag="wv")
        wqkre = wqkT.rearrange("(o p) f -> p o f", p=P)
        wvre = wvT.rearrange("(o p) f -> p o f", p=P)
        TH = T // 2
        for i in range(8):
            nc.sync.dma_start(wqk[:, i, :], wqkre[:, i, :])
            nc.sync.dma_start(xt(i)[:, :TH], xtre[:, i, :TH])
        for i in range(8):
            nc.sync.dma_start(wv[:, i, :], wvre[:, i, :])
            nc.sync.dma_start(xt(i)[:, TH:], xtre[:, i, TH:])
        wp = pers.tile([P, 4, 1024], DT_BF, tag="wp")

        mask = pers.tile([P, P], DT_F32, tag="mask")
        make_causal_mask(nc, mask[:], mask_val=-1e30)

        # ---- stage 1 helpers: qkT [o, t] and V [t, d] per t-chunk ----
        qkT = pers.tile([P, 8, T], DT_BF, tag="qkT")
        v = pers.tile([P, NT, 512], DT_BF, tag="v")

        def qkv_groups(tc4):
            ts = slice(tc4 * 512, (tc4 + 1) * 512)

            def qk_group(oc):
                def go():
                    acc = acc_ps.tile([P, 512], DT_F32, tag="acc")
                    for cc in range(8):
                        nc.tensor.matmul(
                            acc[:],
                            wqk[:, cc, oc * P:(oc + 1) * P],
                            xt(cc)[:, ts],
                            start=(cc == 0), stop=(cc == 7),
                        )
                    nc.vector.tensor_copy(qkT[:, oc, ts], acc[:])
                return go

            def v_group(tt):
                def go():
                    accv = acc_ps.tile([P, 512], DT_F32, tag="acc")
                    for cc in range(8):
                        nc.tensor.matmul(
                            accv[:],
                            xt(cc)[:, tt * P:(tt + 1) * P],
                            wv[:, cc, :],
                            start=(cc == 0), stop=(cc == 7),
                        )
                    nc.vector.tensor_copy(v[:, tt, :], accv[:])
                return go

            # K chunks (oc 4..7) first so QK chains unblock early, then Q, then V
            return ([qk_group(oc) for oc in range(4, 8)]
                    + [qk_group(oc) for oc in range(0, 4)]
                    + [v_group(tt) for tt in range(tc4 * 4, tc4 * 4 + 4)])

        oT = pers.tile([P, 4, T], DT_BF, tag="oT")

        def emit_qk_chain(sup, pr):
            qp = qkT[:, pr, :]        # [128, T]: head a at part 0-63, b at 64-127
            kp = qkT[:, 4 + pr, :]
            ptA = pt4.tile([P, NT, 512], DT_BF, tag="pt4")
            ptB = pt4.tile([P, NT, 512], DT_BF, tag="pt4")
            for it in range(4):
                ti = sup * 4 + it
                jmax = P * (ti + 1)
                isl = slice(ti * P, (ti + 1) * P)
                pA = workp.tile([P, T], DT_BF, tag="pAB")
                pB = workp.tile([P, T], DT_BF, tag="pAB")
                sA = small.tile([P, 2], DT_F32, tag="sA")
                sB = small.tile([P, 2], DT_F32, tag="sB")
                nch = (jmax + 1023) // 1024
                for ch in range(nch):
                    w = min(1024, jmax - ch * 1024)
                    spA = sps.tile([P, 1024], DT_F32, tag="sps")
                    spB = sps.tile([P, 1024], DT_F32, tag="sps")
                    for q5 in range((w + 511) // 512):
                        w5 = min(512, w - q5 * 512)
                        joff = ch * 1024 + q5 * 512
                        jsl = slice(joff, joff + w5)
                        osl = slice(q5 * 512, q5 * 512 + w5)
                        nc.tensor.matmul(spA[:, osl], qp[0:64, isl],
                                         kp[0:64, jsl], start=True, stop=True)
                        nc.tensor.matmul(spB[:, osl], qp[64:128, isl],
                                         kp[64:128, jsl], start=True, stop=True)
                    if ch == nch - 1:
                        moff = (jmax - P) - ch * 1024
                        msl = slice(moff, moff + P)
                        nc.vector.tensor_add(spA[:, msl], spA[:, msl], mask[:])
                        nc.vector.tensor_add(spB[:, msl], spB[:, msl], mask[:])
                    csl = slice(ch * 1024, ch * 1024 + w)
                    nc.scalar.activation(pA[:, csl], spA[:, :w], Exp,
                                         accum_out=sA[:, ch:ch + 1])
                    nc.scalar.activation(pB[:, csl], spB[:, :w], Exp,
                                         accum_out=sB[:, ch:ch + 1])
                rA = small.tile([P, 1], DT_F32, tag="rA")
                rB = small.tile([P, 1], DT_F32, tag="rB")
                if nch == 1:
                    totA, totB = sA[:, 0:1], sB[:, 0:1]
                else:
                    tA = small.tile([P, 1], DT_F32, tag="tA")
                    tB = small.tile([P, 1], DT_F32, tag="tB")
                    nc.vector.tensor_add(tA[:], sA[:, 0:1], sA[:, 1:2])
                    nc.vector.tensor_add(tB[:], sB[:, 0:1], sB[:, 1:2])
                    totA, totB = tA[:], tB[:]
                nc.vector.reciprocal_approx_fast(rA[:], totA)
                nc.vector.reciprocal_approx_fast(rB[:], totB)
                nc.vector.tensor_scalar_mul(pA[:, :jmax], pA[:, :jmax], rA[:])
                nc.vector.tensor_scalar_mul(pB[:, :jmax], pB[:, :jmax], rB[:])
                nc.sync.dma_start_transpose(
                    ptA[:, 0:jmax // P, it * P:(it + 1) * P], pA[:, :jmax])
                nc.sync.dma_start_transpose(
                    ptB[:, 0:jmax // P, it * P:(it + 1) * P], pB[:, :jmax])
            return ptA, ptB

        def emit_av(sup, pr, ptA, ptB):
            av = av_ps.tile([P, 512], DT_F32, tag="av")
            njt = sup * 4 + 4
            for jt in range(njt):
                ioff = max(0, (jt - sup * 4) * P)
                is2 = slice(ioff, 512)
                nc.tensor.matmul(av[0:64, is2], v[:, jt, pr * P:pr * P + 64],
                                 ptA[:, jt, is2],
                                 start=(jt == 0), stop=(jt == njt - 1),
                                 skip_group_check=True)
                nc.tensor.matmul(av[64:128, is2], v[:, jt, pr * P + 64:(pr + 1) * P],
                                 ptB[:, jt, is2],
                                 start=(jt == 0), stop=(jt == njt - 1),
                                 skip_group_check=True)
            nc.vector.tensor_copy(oT[:, pr, sup * 512:(sup + 1) * 512], av[:])

        def proj_groups(sup):
            def group(tt, oc2):
                def go():
                    ysb = worky.tile([P, 512], DT_F32, tag="ysb")
                    acc = acc_ps.tile([P, 512], DT_F32, tag="acc")
                    for hc in range(4):
                        nc.tensor.matmul(
                            acc[:],
                            oT[:, hc, tt * P:(tt + 1) * P],
                            wp[:, hc, oc2 * 512:(oc2 + 1) * 512],
                            start=(hc == 0), stop=(hc == 3),
                        )
                    nc.vector.tensor_copy(ysb[:], acc[:])
                    nc.sync.dma_start(
                        y[tt * P:(tt + 1) * P, oc2 * 512:(oc2 + 1) * 512], ysb[:])
                return go
            return [group(tt, oc2)
                    for tt in range(sup * 4, sup * 4 + 4) for oc2 in range(2)]

        # Fused, finely woven emission: attention chains for super `sup`
        # interleave with matmul groups of the next qkv chunk and the previous
        # super's projection, keeping PE fed while ACT/DVE/DMA drain softmax.
        wpre = wpT.rearrange("(o p) f -> p o f", p=P)
        for sup in range(NSUP):
            if sup == 0:
                # qkv(0) woven as pre-chain fillers (each chain's Q/K chunks
                # land just before it, V before the first AV); qkv(1) follows
                # as post-filler
                g0 = qkv_groups(0)  # [oc4..7, oc0..3, v0..3]
                k_, q_, v_ = g0[0:4], g0[4:8], g0[8:12]
                pre = [[k_[0], q_[0]],
                       v_ + [k_[1], q_[1]],
                       [k_[2], q_[2]],
                       [k_[3], q_[3]]]
                filler = qkv_groups(1)
            else:
                pre = [[], [], [], []]
                filler = qkv_groups(sup + 1) if sup < NSUP - 1 else []
                filler = filler + proj_groups(sup - 1)
            nfil = (len(filler) + NPAIR - 1) // NPAIR if filler else 0
            pts = {}
            for pr in range(NPAIR):
                for g in pre[pr]:
                    g()
                pts[pr] = emit_qk_chain(sup, pr)
                if pr >= 1:
                    emit_av(sup, pr - 1, *pts.pop(pr - 1))
                for g in filler[pr * nfil:(pr + 1) * nfil]:
                    g()
            emit_av(sup, NPAIR - 1, *pts.pop(NPAIR - 1))
            if sup == 0:
                for i in range(4):
                    nc.sync.dma_start(wp[:, i, :], wpre[:, i, :])
        for g in proj_groups(NSUP - 1):
            g()

    nc.compile()
    return nc


def _prep_inputs(x, W_qkv, W_proj):
    """Per-core host-side sharding and layout prep."""
    bf16 = ml_dtypes.bfloat16
    scale = np.float32(HD ** -0.5)
    in_maps = []
    for c in range(NCORES):
        b, hg = c // 2, c % 2
        heads = list(range(hg * 8, hg * 8 + 8))
        rq = np.concatenate([np.arange(h * 192, h * 192 + 64) for h in heads])
        rk = np.concatenate([np.arange(h * 192 + 64, h * 192 + 128) for h in heads])
        rv = np.concatenate([np.arange(h * 192 + 128, h * 192 + 192) for h in heads])
        wq = W_qkv[rq] * scale           # fold softmax scale into Q (exact: /8)
        wk = W_qkv[rk]
        wqkT = np.ascontiguousarray(np.concatenate([wq, wk], 0).T).astype(bf16)
        wvT = np.ascontiguousarray(W_qkv[rv].T).astype(bf16)
        wpT = np.ascontiguousarray(W_proj[:, hg * 512:(hg + 1) * 512].T)
        xTb = np.ascontiguousarray(x[b].T).astype(bf16)
        in_maps.append({"xT": xTb, "wqkT": wqkT, "wvT": wvT,
                        "wpT": wpT.astype(bf16)})
    return in_maps


def kernel(x, W_qkv, W_proj, b_proj):
    from concourse.bass_utils import run_bass_kernel_spmd

    x = np.asarray(x, dtype=np.float32)
    W_qkv = np.asarray(W_qkv, dtype=np.float32)
    W_proj = np.asarray(W_proj, dtype=np.float32)
    b_proj = np.asarray(b_proj, dtype=np.float32)

    if "nc" not in _CACHE:
        _CACHE["nc"] = _build_program()
    nc = _CACHE["nc"]

    in_maps = _prep_inputs(x, W_qkv, W_proj)
    res = run_bass_kernel_spmd(nc, in_maps, core_ids=list(range(NCORES)))
    out = np.empty((B, T, C), dtype=np.float32)
    for b in range(B):
        out[b] = res.results[2 * b]["y"] + res.results[2 * b + 1]["y"] + b_proj
    return out



# revision 7
# speedup vs baseline: 1.1426x; 1.1426x over previous
"""Multi-head causal attention (B=4, T=2048, C=1024, H=16, HD=64) on 8 TRN2 NeuronCores.

Sharding: core c handles batch b = c//2 and heads hg*8..hg*8+8 where hg = c%2
(data parallel on B, tensor parallel on heads). Each core computes
qkv projection for its head group, causal attention for its 8 heads, and a
partial output projection over its 512 local channels. Host sums the two
partial projections per batch and adds the bias.

Per-core device layouts (all chosen so no on-chip transposes of x/W are needed):
  xT    [C=1024, T=2048] bf16   (x[b].T, host-transposed)
  wqkT  [C=1024, 1024]   bf16   (rows: Q of 8 heads then K of 8 heads, transposed;
                                 Q part pre-scaled by HD^-0.5)
  wvT   [C=1024, 512]    bf16
  wpT   [512, 1024]      bf16   (W_proj columns for local channels, transposed)
Stages:
  1. qkT[o, t] = W_sel @ x.T  (o: 512 Q + 512 K, head-pair h'=2p at partitions
     0-63 / 64-127 of chunk p)  and V [t, (h, d+ones)] natural with a ones
     column appended per head.
  2. Attention computes S^T directly: for each (pair, i-super of 512, j-tile
     of 128): S^T[j, i] = K_tile^T Q (row-tiled K=64 matmul pair), causal mask
     add on the diagonal block, exp on ACT (no accumulation needed) giving
     P^T[j, i] in SBUF bf16. AV accumulates O[i, d] (+ row-sum denominator via
     the ones column) with cheap N=65 matmuls: O_psum[i-tile] += P^T_tile^T
     @ [V|1]. After the j-loop, rows are normalized by the reciprocal of the
     denominator on DVE and O is DMA-transposed ([128 t, 2x64 hd] blocks) into
     oT[d, t] for the projection.
  3. y[t, :] = O @ WpT (bf16) accumulated in fp32 PSUM.
"""

import numpy as np
import ml_dtypes

B, T, C = 4, 2048, 1024
H = 16
HD = 64
NCORES = 8
P = 128

_CACHE = {}


def _build_program():
    import concourse.bass as bass
    import concourse.mybir as mybir
    import concourse.tile as tile
    from concourse import bacc
    from contextlib import ExitStack

    DT_BF = mybir.dt.bfloat16
    DT_F32 = mybir.dt.float32
    Exp = mybir.ActivationFunctionType.Exp

    nc = bacc.Bacc("TRN2", target_bir_lowering=False, num_devices=NCORES)
    xT = nc.dram_tensor("xT", [C, T], DT_BF, kind="ExternalInput")
    wqkT = nc.dram_tensor("wqkT", [C, 1024], DT_BF, kind="ExternalInput")
    wvT = nc.dram_tensor("wvT", [C, 512], DT_BF, kind="ExternalInput")
    wpT = nc.dram_tensor("wpT", [512, 1024], DT_BF, kind="ExternalInput")
    y = nc.dram_tensor("y", [T, C], DT_F32, kind="ExternalOutput")

    NT = T // P            # 16 t-tiles
    NSUP = 4               # i-supers of 512
    NPAIR = 4              # head pairs per core
    LAG = 2                # AV trails exp by this many j-tiles

    with tile.TileContext(nc) as tc, ExitStack() as ctx:
        pers = ctx.enter_context(tc.tile_pool(name="pers", bufs=1))
        xtp = ctx.enter_context(tc.tile_pool(name="xtp", bufs=2))
        ptp = ctx.enter_context(tc.tile_pool(name="ptp", bufs=2))
        obp = ctx.enter_context(tc.tile_pool(name="obp", bufs=3))
        worky = ctx.enter_context(tc.tile_pool(name="worky", bufs=2))
        small = ctx.enter_context(tc.tile_pool(name="small", bufs=8))
        sps = ctx.enter_context(tc.tile_pool(name="sps", bufs=2, space="PSUM"))
        avp = ctx.enter_context(tc.tile_pool(name="av", bufs=4, space="PSUM"))
        accp = ctx.enter_context(tc.tile_pool(name="acc", bufs=2, space="PSUM"))

        # ---- loads ----
        xt0 = xtp.tile([P, 4, T], DT_BF, tag="xtp")
        xt1 = xtp.tile([P, 4, T], DT_BF, tag="xtp")
        xtre = xT.rearrange("(o p) t -> p o t", p=P)

        def xt(cc):
            return (xt0 if cc < 4 else xt1)[:, cc % 4, :]
        wqk = pers.tile([P, 8, 1024], DT_BF, tag="wqk")
        wv = pers.tile([P, 8, 512], DT_BF, tag="wv")
        wqkre = wqkT.rearrange("(o p) f -> p o f", p=P)
        wvre = wvT.rearrange("(o p) f -> p o f", p=P)
        TH = T // 2
        for i in range(8):
            nc.sync.dma_start(wqk[:, i, :], wqkre[:, i, :])
            nc.sync.dma_start(xt(i)[:, :TH], xtre[:, i, :TH])
        for i in range(8):
            nc.sync.dma_start(wv[:, i, :], wvre[:, i, :])
            nc.sync.dma_start(xt(i)[:, TH:], xtre[:, i, TH:])
        wp = pers.tile([P, 4, 1024], DT_BF, tag="wp")

        # maskT[j, i] = -1e30 where j > i (S^T orientation: partition=j, free=i)
        maskT = pers.tile([P, P], DT_F32, tag="maskT")
        nc.gpsimd.memset(maskT[:], 0.0)
        nc.gpsimd.affine_select(
            out=maskT[:],
            in_=maskT[:],
            compare_op=mybir.AluOpType.is_ge,
            fill=-1e30,
            base=0,
            # keep (i - j) >= 0, i.e. j <= i
            pattern=[[1, P]],
            channel_multiplier=-1,
        )

        # ---- stage 1: qkT [o, t] and V [t, (h, d|1)] per t-chunk ----
        qkT = pers.tile([P, 8, T], DT_BF, tag="qkT")
        v2 = pers.tile([P, NT, 8, 65], DT_BF, tag="v2")
        nc.gpsimd.memset(v2[:], 1.0)

        def qkv_groups(tc4):
            ts = slice(tc4 * 512, (tc4 + 1) * 512)

            def qk_group(oc):
                def go():
                    acc = accp.tile([P, 512], DT_F32, tag="acc")
                    for cc in range(8):
                        nc.tensor.matmul(
                            acc[:],
                            wqk[:, cc, oc * P:(oc + 1) * P],
                            xt(cc)[:, ts],
                            start=(cc == 0), stop=(cc == 7),
                        )
                    nc.vector.tensor_copy(qkT[:, oc, ts], acc[:])
                return go

            def v_group(tt):
                def go():
                    accv = accp.tile([P, 512], DT_F32, tag="acc")
                    for cc in range(8):
                        nc.tensor.matmul(
                            accv[:],
                            xt(cc)[:, tt * P:(tt + 1) * P],
                            wv[:, cc, :],
                            start=(cc == 0), stop=(cc == 7),
                        )
                    nc.vector.tensor_copy(
                        v2[:, tt, :, 0:64],
                        accv[:].rearrange("p (h d) -> p h d", d=64))
                return go

            return ([qk_group(oc) for oc in range(4, 8)]
                    + [qk_group(oc) for oc in range(0, 4)]
                    + [v_group(tt) for tt in range(tc4 * 4, tc4 * 4 + 4)])

        oT = pers.tile([P, 4, T], DT_BF, tag="oT")

        def chain(sup, pr, fillers):
            """Attention for head pair pr, query rows [sup*512, (sup+1)*512)."""
            qp = qkT[:, pr, :]        # [128, T]: head A at part 0-63, B at 64-127
            kp = qkT[:, 4 + pr, :]
            i0 = sup * 512
            njt = 4 * sup + 4
            oH = [avp.tile([P, 512], DT_F32, tag="av", name=f"oH{sup}_{pr}_{h}")
                  for h in range(2)]
            pTall = ptp.tile([P, njt, 2, 512], DT_BF, tag="ptp",
                             name=f"pTall{sup}_{pr}")
            nfil = len(fillers)
            fi = 0

            for jt in range(njt):
                # paced filler (before the gated QK so PE has queued work)
                while fi * njt < (jt + 1) * nfil:
                    fillers[fi]()
                    fi += 1
                ext_start = max(i0, jt * P)
                ext = i0 + 512 - ext_start
                sp2 = [sps.tile([P, 512], DT_F32, tag="sps", name=f"sp{jt}_{h}")
                       for h in range(2)]
                for hh, sp in enumerate(sp2):
                    hsl = slice(hh * 64, hh * 64 + 64)
                    nc.tensor.matmul(
                        sp[:, :ext],
                        kp[hsl, jt * P:(jt + 1) * P],
                        qp[hsl, ext_start:ext_start + ext],
                        start=True, stop=True,
                    )
                    if jt >= 4 * sup:  # diagonal block at chunk cols 0:128
                        nc.vector.tensor_add(sp[:, 0:P], sp[:, 0:P], maskT[:])
                    nc.scalar.activation(pTall[:, jt, hh, :ext], sp[:, :ext], Exp)
            while fi < nfil:
                fillers[fi]()
                fi += 1
            # AV: per (i-tile, head) a contiguous accumulation group over j.
            # Groups sharing a PSUM bank must not interleave (start=True marks
            # the whole 2KB zero region), so bursts run group-by-group.
            for itl in range(4):
                itg = sup * 4 + itl
                for hh in range(2):
                    for jt in range(itg + 1):
                        ext_start = max(i0, jt * P)
                        off = i0 + itl * P - ext_start
                        nc.tensor.matmul(
                            oH[hh][:, itl * P:itl * P + 65],
                            pTall[:, jt, hh, off:off + P],
                            v2[:, jt, 2 * pr + hh, :],
                            start=(jt == 0), stop=(jt == itg),
                        )

            # normalize rows by the ones-column denominator, pack for transpose
            rc = small.tile([P, 8], DT_F32, tag="rc")
            ob = obp.tile([P, 8, 64], DT_BF, tag="ob")  # [t, (itl, h), d]
            for hh in range(2):
                den = oH[hh][:].rearrange("p (i c) -> p i c", c=P)[:, :, 64]
                nc.vector.reciprocal_approx_fast(rc[:, hh * 4:hh * 4 + 4], den)
                for itl in range(4):
                    nc.vector.tensor_scalar_mul(
                        ob[:, itl * 2 + hh, :],
                        oH[hh][:, itl * P:itl * P + 64],
                        rc[:, hh * 4 + itl:hh * 4 + itl + 1])
            for itl in range(4):
                nc.sync.dma_start_transpose(
                    oT[:, pr, i0 + itl * P:i0 + (itl + 1) * P],
                    ob[:, itl * 2:itl * 2 + 2, :])

        def proj_groups(sup):
            def group(tt, oc2):
                def go():
                    ysb = worky.tile([P, 512], DT_F32, tag="ysb")
                    acc = accp.tile([P, 512], DT_F32, tag="acc")
                    for hc in range(4):
                        nc.tensor.matmul(
                            acc[:],
                            oT[:, hc, tt * P:(tt + 1) * P],
                            wp[:, hc, oc2 * 512:(oc2 + 1) * 512],
                            start=(hc == 0), stop=(hc == 3),
                        )
                    nc.vector.tensor_copy(ysb[:], acc[:])
                    nc.sync.dma_start(
                        y[tt * P:(tt + 1) * P, oc2 * 512:(oc2 + 1) * 512], ysb[:])
                return go
            return [group(tt, oc2)
                    for tt in range(sup * 4, sup * 4 + 4) for oc2 in range(2)]

        wpre = wpT.rearrange("(o p) f -> p o f", p=P)
        for sup in range(NSUP):
            if sup == 0:
                g0 = qkv_groups(0)  # [k4..7, q0..3, v0..3]
                k_, q_, v_ = g0[0:4], g0[4:8], g0[8:12]
                pre = [[k_[0], q_[0]],
                       [k_[1], q_[1]],
                       [k_[2], q_[2]],
                       [k_[3], q_[3]]]
                # v0..3 must be chain(0,0)'s fillers: its AV burst reads all
                # four v2 chunks, and fillers flush before the burst.
                filler = v_ + qkv_groups(1)
            else:
                pre = [[], [], [], []]
                filler = qkv_groups(sup + 1) if sup < NSUP - 1 else []
                filler = filler + proj_groups(sup - 1)
            nfil = (len(filler) + NPAIR - 1) // NPAIR if filler else 0
            for pr in range(NPAIR):
                for g in pre[pr]:
                    g()
                chain(sup, pr, filler[pr * nfil:(pr + 1) * nfil])
            if sup == 0:
                for i in range(4):
                    nc.sync.dma_start(wp[:, i, :], wpre[:, i, :])
        for g in proj_groups(NSUP - 1):
            g()

    nc.compile()
    return nc


def _prep_inputs(x, W_qkv, W_proj):
    """Per-core host-side sharding and layout prep."""
    bf16 = ml_dtypes.bfloat16
    scale = np.float32(HD ** -0.5)
    in_maps = []
    for c in range(NCORES):
        b, hg = c // 2, c % 2
        heads = list(range(hg * 8, hg * 8 + 8))
        rq = np.concatenate([np.arange(h * 192, h * 192 + 64) for h in heads])
        rk = np.concatenate([np.arange(h * 192 + 64, h * 192 + 128) for h in heads])
        rv = np.concatenate([np.arange(h * 192 + 128, h * 192 + 192) for h in heads])
        wq = W_qkv[rq] * scale           # fold softmax scale into Q (exact: /8)
        wk = W_qkv[rk]
        wqkT = np.ascontiguousarray(np.concatenate([wq, wk], 0).T).astype(bf16)
        wvT = np.ascontiguousarray(W_qkv[rv].T).astype(bf16)
        wpT = np.ascontiguousarray(W_proj[:, hg * 512:(hg + 1) * 512].T)
        xTb = np.ascontiguousarray(x[b].T).astype(bf16)
        in_maps.append({"xT": xTb, "wqkT": wqkT, "wvT": wvT,
                        "wpT": wpT.astype(bf16)})
    return in_maps


def kernel(x, W_qkv, W_proj, b_proj):
    from concourse.bass_utils import run_bass_kernel_spmd

    x = np.asarray(x, dtype=np.float32)
    W_qkv = np.asarray(W_qkv, dtype=np.float32)
    W_proj = np.asarray(W_proj, dtype=np.float32)
    b_proj = np.asarray(b_proj, dtype=np.float32)

    if "nc" not in _CACHE:
        _CACHE["nc"] = _build_program()
    nc = _CACHE["nc"]

    in_maps = _prep_inputs(x, W_qkv, W_proj)
    res = run_bass_kernel_spmd(nc, in_maps, core_ids=list(range(NCORES)))
    out = np.empty((B, T, C), dtype=np.float32)
    for b in range(B):
        out[b] = res.results[2 * b]["y"] + res.results[2 * b + 1]["y"] + b_proj
    return out


# revision 12
# speedup vs baseline: 1.2139x; 1.0624x over previous
"""Multi-head causal attention (B=4, T=2048, C=1024, H=16, HD=64) on 8 TRN2 NeuronCores.

Sharding: core c handles batch b = c//2 and heads hg*8..hg*8+8 where hg = c%2
(data parallel on B, tensor parallel on heads). Each core computes
qkv projection for its head group, causal attention for its 8 heads, and a
partial output projection over its 512 local channels. Host sums the two
partial projections per batch and adds the bias.

Per-core device layouts (all chosen so no on-chip transposes of x/W are needed):
  xT    [C=1024, T=2048] bf16   (x[b].T, host-transposed)
  wqkT  [C=1024, 1024]   bf16   (rows: Q of 8 heads then K of 8 heads, transposed;
                                 Q part pre-scaled by HD^-0.5)
  wvT   [C=1024, 512]    bf16
  wpT   [512, 1024]      bf16   (W_proj columns for local channels, transposed)
Stages:
  1. qkT[o, t] = W_sel @ x.T  (o: 512 Q + 512 K, head-pair h'=2p at partitions
     0-63 / 64-127 of chunk p)  and V [t, (h, d+ones)] natural with a ones
     column appended per head.
  2. Attention computes S^T directly: for each (pair, i-super of 512, j-tile
     of 128): S^T[j, i] = K_tile^T Q (row-tiled K=64 matmul pair), causal mask
     add on the diagonal block, exp on ACT (no accumulation needed) giving
     P^T[j, i] in SBUF bf16. AV accumulates O[i, d] (+ row-sum denominator via
     the ones column) with cheap N=65 matmuls: O_psum[i-tile] += P^T_tile^T
     @ [V|1]. After the j-loop, rows are normalized by the reciprocal of the
     denominator on DVE and O is DMA-transposed ([128 t, 2x64 hd] blocks) into
     oT[d, t] for the projection.
  3. y[t, :] = O @ WpT (bf16) accumulated in fp32 PSUM.
"""

import numpy as np
import ml_dtypes

B, T, C = 4, 2048, 1024
H = 16
HD = 64
NCORES = 8
P = 128

_CACHE = {}


def _build_program():
    import concourse.bass as bass
    import concourse.mybir as mybir
    import concourse.tile as tile
    from concourse import bacc
    from contextlib import ExitStack

    DT_BF = mybir.dt.bfloat16
    DT_F32 = mybir.dt.float32
    Exp = mybir.ActivationFunctionType.Exp

    nc = bacc.Bacc("TRN2", target_bir_lowering=False, num_devices=NCORES)
    xT = nc.dram_tensor("xT", [C, T], DT_BF, kind="ExternalInput")
    wqkT = nc.dram_tensor("wqkT", [C, 1024], DT_BF, kind="ExternalInput")
    wvT = nc.dram_tensor("wvT", [C, 512], DT_BF, kind="ExternalInput")
    wpT = nc.dram_tensor("wpT", [512, 1024], DT_BF, kind="ExternalInput")
    y = nc.dram_tensor("y", [T, C], DT_F32, kind="ExternalOutput")

    NT = T // P            # 16 t-tiles
    NSUP = 4               # i-supers of 512
    NPAIR = 4              # head pairs per core
    LAG = 2                # AV trails exp by this many j-tiles

    with tile.TileContext(nc) as tc, ExitStack() as ctx:
        pers = ctx.enter_context(tc.tile_pool(name="pers", bufs=1))
        xtp = ctx.enter_context(tc.tile_pool(name="xtp", bufs=2))
        ptp = ctx.enter_context(tc.tile_pool(name="ptp", bufs=2))
        obp = ctx.enter_context(tc.tile_pool(name="obp", bufs=3))
        worky = ctx.enter_context(tc.tile_pool(name="worky", bufs=2))
        small = ctx.enter_context(tc.tile_pool(name="small", bufs=8))
        sps = ctx.enter_context(tc.tile_pool(name="sps", bufs=2, space="PSUM"))
        avp = ctx.enter_context(tc.tile_pool(name="av", bufs=2, space="PSUM"))
        accp = ctx.enter_context(tc.tile_pool(name="acc", bufs=2, space="PSUM"))

        # ---- loads ----
        xt0 = xtp.tile([P, 4, T], DT_BF, tag="xtp")
        xt1 = xtp.tile([P, 4, T], DT_BF, tag="xtp")
        xtre = xT.rearrange("(o p) t -> p o t", p=P)

        def xt(cc):
            return (xt0 if cc < 4 else xt1)[:, cc % 4, :]
        wqk = pers.tile([P, 8, 1024], DT_BF, tag="wqk")
        wv = pers.tile([P, 8, 512], DT_BF, tag="wv")
        wqkre = wqkT.rearrange("(o p) f -> p o f", p=P)
        wvre = wvT.rearrange("(o p) f -> p o f", p=P)
        TH = T // 2
        for i in range(8):
            nc.sync.dma_start(wqk[:, i, :], wqkre[:, i, :])
            nc.sync.dma_start(xt(i)[:, :TH], xtre[:, i, :TH])
        for i in range(8):
            nc.sync.dma_start(wv[:, i, :], wvre[:, i, :])
            nc.sync.dma_start(xt(i)[:, TH:], xtre[:, i, TH:])
        wp = pers.tile([P, 4, 1024], DT_BF, tag="wp")

        # maskT2[j, hh, i] = -1e30 where j > i (S^T orientation: partition=j,
        # free=i), replicated for both heads so one DVE add masks both.
        maskT2 = pers.tile([P, 2, P], DT_F32, tag="maskT2")
        nc.gpsimd.memset(maskT2[:], 0.0)
        for hh in range(2):
            nc.gpsimd.affine_select(
                out=maskT2[:, hh, :],
                in_=maskT2[:, hh, :],
                compare_op=mybir.AluOpType.is_ge,
                fill=-1e30,
                base=0,
                # keep (i - j) >= 0, i.e. j <= i
                pattern=[[1, P]],
                channel_multiplier=-1,
            )

        # ---- stage 1: qkT [o, t] and V [t, (h, d|1)] per t-chunk ----
        qkT = pers.tile([P, 8, T], DT_BF, tag="qkT")
        v2 = pers.tile([P, NT, 8, 65], DT_BF, tag="v2")
        nc.gpsimd.memset(v2[:], 1.0)

        def qkv_groups(tc4):
            ts = slice(tc4 * 512, (tc4 + 1) * 512)

            def qk_group(oc):
                def go():
                    acc = accp.tile([P, 512], DT_F32, tag="acc")
                    for cc in range(8):
                        nc.tensor.matmul(
                            acc[:],
                            wqk[:, cc, oc * P:(oc + 1) * P],
                            xt(cc)[:, ts],
                            start=(cc == 0), stop=(cc == 7),
                        )
                    nc.vector.tensor_copy(qkT[:, oc, ts], acc[:])
                return go

            def v_group(tt):
                def go():
                    accv = accp.tile([P, 512], DT_F32, tag="acc")
                    for cc in range(8):
                        nc.tensor.matmul(
                            accv[:],
                            xt(cc)[:, tt * P:(tt + 1) * P],
                            wv[:, cc, :],
                            start=(cc == 0), stop=(cc == 7),
                        )
                    nc.vector.tensor_copy(
                        v2[:, tt, :, 0:64],
                        accv[:].rearrange("p (h d) -> p h d", d=64))
                return go

            return ([qk_group(oc) for oc in range(4, 8)]
                    + [qk_group(oc) for oc in range(0, 4)]
                    + [v_group(tt) for tt in range(tc4 * 4, tc4 * 4 + 4)])

        oT = pers.tile([P, 4, T], DT_BF, tag="oT")

        def chain(sup, pr, fillers):
            """Attention for head pair pr, query rows [sup*512, (sup+1)*512)."""
            qp = qkT[:, pr, :]        # [128, T]: head A at part 0-63, B at 64-127
            kp = qkT[:, 4 + pr, :]
            i0 = sup * 512
            njt = 4 * sup + 4
            oH = [avp.tile([P, 512], DT_F32, tag="av", name=f"oH{sup}_{pr}_{h}")
                  for h in range(2)]
            pTall = ptp.tile([P, njt, 2, 512], DT_BF, tag="ptp",
                             name=f"pTall{sup}_{pr}")
            nfil = len(fillers)
            fi = 0

            for jt in range(njt):
                # paced filler (before the gated QK so PE has queued work)
                while fi * njt < (jt + 1) * nfil:
                    fillers[fi]()
                    fi += 1
                ext_start = max(i0, jt * P)
                ext = i0 + 512 - ext_start
                sp = sps.tile([P, 2, 512], DT_F32, tag="sps", name=f"sp{jt}")
                for hh in range(2):
                    hsl = slice(hh * 64, hh * 64 + 64)
                    nc.tensor.matmul(
                        sp[:, hh, :ext],
                        kp[hsl, jt * P:(jt + 1) * P],
                        qp[hsl, ext_start:ext_start + ext],
                        start=True, stop=True,
                    )
                if jt >= 4 * sup:  # diagonal block at chunk cols 0:128
                    nc.vector.tensor_add(sp[:, :, 0:P], sp[:, :, 0:P], maskT2[:])
                nc.scalar.activation(pTall[:, jt, :, :ext], sp[:, :, :ext], Exp)
            while fi < nfil:
                fillers[fi]()
                fi += 1
            # AV: per (i-tile, head) a contiguous accumulation group over j.
            # Groups sharing a PSUM bank must not interleave (start=True marks
            # the whole 2KB zero region), so bursts run group-by-group.
            for itl in range(4):
                itg = sup * 4 + itl
                for hh in range(2):
                    for jt in range(itg + 1):
                        ext_start = max(i0, jt * P)
                        off = i0 + itl * P - ext_start
                        nc.tensor.matmul(
                            oH[hh][:, itl * P:itl * P + 65],
                            pTall[:, jt, hh, off:off + P],
                            v2[:, jt, 2 * pr + hh, :],
                            start=(jt == 0), stop=(jt == itg),
                        )

            # normalize rows by the ones-column denominator, pack for transpose
            rc = small.tile([P, 8], DT_F32, tag="rc")
            ob = obp.tile([P, 8, 64], DT_BF, tag="ob")  # [t, (itl, h), d]
            for hh in range(2):
                den = oH[hh][:].rearrange("p (i c) -> p i c", c=P)[:, :, 64]
                nc.vector.reciprocal_approx_fast(rc[:, hh * 4:hh * 4 + 4], den)
                for itl in range(4):
                    nc.vector.tensor_scalar_mul(
                        ob[:, itl * 2 + hh, :],
                        oH[hh][:, itl * P:itl * P + 64],
                        rc[:, hh * 4 + itl:hh * 4 + itl + 1])
            for itl in range(4):
                nc.sync.dma_start_transpose(
                    oT[:, pr, i0 + itl * P:i0 + (itl + 1) * P],
                    ob[:, itl * 2:itl * 2 + 2, :])

        def proj_groups(sup):
            def group(tt, oc2):
                def go():
                    ysb = worky.tile([P, 512], DT_F32, tag="ysb")
                    acc = accp.tile([P, 512], DT_F32, tag="acc")
                    for hc in range(4):
                        nc.tensor.matmul(
                            acc[:],
                            oT[:, hc, tt * P:(tt + 1) * P],
                            wp[:, hc, oc2 * 512:(oc2 + 1) * 512],
                            start=(hc == 0), stop=(hc == 3),
                        )
                    nc.vector.tensor_copy(ysb[:], acc[:])
                    nc.gpsimd.dma_start(
                        y[tt * P:(tt + 1) * P, oc2 * 512:(oc2 + 1) * 512], ysb[:])
                return go
            return [group(tt, oc2)
                    for tt in range(sup * 4, sup * 4 + 4) for oc2 in range(2)]

        wpre = wpT.rearrange("(o p) f -> p o f", p=P)
        for sup in range(NSUP):
            if sup == 0:
                g0 = qkv_groups(0)  # [k4..7, q0..3, v0..3]
                k_, q_, v_ = g0[0:4], g0[4:8], g0[8:12]
                pre = [[k_[0], q_[0]],
                       [k_[1], q_[1]],
                       [k_[2], q_[2]],
                       [k_[3], q_[3]]]
                # v0..3 must be chain(0,0)'s fillers: its AV burst reads all
                # four v2 chunks, and fillers flush before the burst.
                filler = v_ + qkv_groups(1)
            else:
                pre = [[], [], [], []]
                filler = qkv_groups(sup + 1) if sup < NSUP - 1 else []
                filler = filler + proj_groups(sup - 1)
            nfil = (len(filler) + NPAIR - 1) // NPAIR if filler else 0
            for pr in range(NPAIR):
                for g in pre[pr]:
                    g()
                chain(sup, pr, filler[pr * nfil:(pr + 1) * nfil])
            if sup == 0:
                for i in range(4):
                    nc.sync.dma_start(wp[:, i, :], wpre[:, i, :])
        for g in proj_groups(NSUP - 1):
            g()

    nc.compile()
    return nc


def _prep_inputs(x, W_qkv, W_proj):
    """Per-core host-side sharding and layout prep."""
    bf16 = ml_dtypes.bfloat16
    scale = np.float32(HD ** -0.5)
    in_maps = []
    for c in range(NCORES):
        b, hg = c // 2, c % 2
        heads = list(range(hg * 8, hg * 8 + 8))
        rq = np.concatenate([np.arange(h * 192, h * 192 + 64) for h in heads])
        rk = np.concatenate([np.arange(h * 192 + 64, h * 192 + 128) for h in heads])
        rv = np.concatenate([np.arange(h * 192 + 128, h * 192 + 192) for h in heads])
        wq = W_qkv[rq] * scale           # fold softmax scale into Q (exact: /8)
        wk = W_qkv[rk]
        wqkT = np.ascontiguousarray(np.concatenate([wq, wk], 0).T).astype(bf16)
        wvT = np.ascontiguousarray(W_qkv[rv].T).astype(bf16)
        wpT = np.ascontiguousarray(W_proj[:, hg * 512:(hg + 1) * 512].T)
        xTb = np.ascontiguousarray(x[b].T).astype(bf16)
        in_maps.append({"xT": xTb, "wqkT": wqkT, "wvT": wvT,
                        "wpT": wpT.astype(bf16)})
    return in_maps


def kernel(x, W_qkv, W_proj, b_proj):
    from concourse.bass_utils import run_bass_kernel_spmd

    x = np.asarray(x, dtype=np.float32)
    W_qkv = np.asarray(W_qkv, dtype=np.float32)
    W_proj = np.asarray(W_proj, dtype=np.float32)
    b_proj = np.asarray(b_proj, dtype=np.float32)

    if "nc" not in _CACHE:
        _CACHE["nc"] = _build_program()
    nc = _CACHE["nc"]

    in_maps = _prep_inputs(x, W_qkv, W_proj)
    res = run_bass_kernel_spmd(nc, in_maps, core_ids=list(range(NCORES)))
    out = np.empty((B, T, C), dtype=np.float32)
    for b in range(B):
        out[b] = res.results[2 * b]["y"] + res.results[2 * b + 1]["y"] + b_proj
    return out
